# revision 1
# baseline (speedup 1.0000x reference)
"""BigramLM embedding lookup as a distributed DMA row-gather.

Z[b,s,:] = W[inputs[b,s],:] -- the one-hot matmul in the reference is just a
row gather from a 256 MB table. Strategy: pure data parallelism over the
8*512=4096 tokens; each of the 8 cores owns 512 tokens and gathers their rows
from its full local copy of W with SWDGE indirect DMA (HBM->SBUF), while
HWDGE stores finished chunks to the output (SBUF->HBM).

Chunk k slot p holds token k*CHUNK+p in SBUF partition p; the store writes
partition p to output row k*CHUNK+p, so the output comes back in natural
token order with no device-side reorder. One SBUF buffer per chunk -- no
buffer-reuse dependencies, all gathers are queued back to back.
"""

from contextlib import ExitStack

import numpy as np

import concourse.bacc as bacc
import concourse.bass as bass
import concourse.mybir as mybir
from concourse.bass_utils import run_bass_kernel_spmd

VOCAB = 8192
EMB = 8192
BATCH, SEQ = 8, 512
N_CORES = 8
TOK = BATCH * SEQ // N_CORES  # 512 tokens per core
CHUNK = 128                   # tokens per gather chunk (= SBUF partitions)
K = TOK // CHUNK              # chunks per core

_cache: dict = {}

# Results object of the most recent run (test.py reads exec_time_ns off it).
LAST_RESULTS = None


def _build():
    nc = bacc.Bacc("TRN2", enable_partition_id=False, monotonic_sem_count=0)
    w = nc.dram_tensor("w", [VOCAB, EMB], mybir.dt.float32, kind="ExternalInput")
    idx = nc.dram_tensor("idx", [CHUNK, K], mybir.dt.int32, kind="ExternalInput")
    out = nc.dram_tensor("out", [TOK, EMB], mybir.dt.float32, kind="ExternalOutput")
    with (
        nc.Block() as block,
        ExitStack() as stack,
        nc.semaphore("io") as io,
        nc.semaphore("gsem") as gsem,
        nc.semaphore("ssem") as ssem,
    ):
        idx_sb = stack.enter_context(
            nc.sbuf_tensor("idx_sb", [CHUNK, K], mybir.dt.int32)
        )
        bufs = [
            stack.enter_context(
                nc.sbuf_tensor(f"buf{k}", [CHUNK, EMB], mybir.dt.float32)
            )
            for k in range(K)
        ]

        @block.gpsimd
        def _(gp):
            gp.wait_ge(io, 16)
            for k in range(K):
                gp.indirect_dma_start(
                    out=bufs[k][:],
                    out_offset=None,
                    in_=w[:],
                    in_offset=bass.IndirectOffsetOnAxis(
                        ap=idx_sb[:, k : k + 1], axis=0
                    ),
                ).then_inc(gsem, 16)

        @block.sync
        def _(sy):
            sy.dma_start(idx_sb[:], idx[:]).then_inc(io, 16)
            for k in range(K):
                sy.wait_ge(gsem, 16 * (k + 1))
                sy.dma_start(
                    out[k * CHUNK : (k + 1) * CHUNK, :],
                    bufs[k][:],
                    single_packet=True,
                ).then_inc(ssem, 16)
            sy.wait_ge(ssem, 16 * K)

    nc.compile()
    return nc


def kernel(inputs, W):
    global LAST_RESULTS
    inputs = np.asarray(inputs)
    W = np.ascontiguousarray(np.asarray(W, dtype=np.float32))
    flat = inputs.reshape(-1).astype(np.int64)
    assert flat.shape == (N_CORES * TOK,)
    assert flat.min() >= 0 and flat.max() < VOCAB

    nc = _cache.get("nc")
    if nc is None:
        nc = _cache["nc"] = _build()

    in_maps = []
    for c in range(N_CORES):
        tok = flat[c * TOK : (c + 1) * TOK]
        # chunk k slot p = token k*CHUNK+p -> idx_sb[p, k]
        idx2d = np.ascontiguousarray(
            tok.reshape(K, CHUNK).T.astype(np.int32)
        )
        in_maps.append({"w": W, "idx": idx2d})
    res = run_bass_kernel_spmd(nc, in_maps, core_ids=list(range(N_CORES)))
    LAST_RESULTS = res
    outs = [res.results[c]["out"] for c in range(N_CORES)]
    return np.concatenate(outs, axis=0).reshape(BATCH, SEQ, EMB)

